# revision 21
# baseline (speedup 1.0000x reference)
"""DGCNN forward kernel for Trainium2 (8 NeuronCores, pure data parallel over batch).

Each core processes one sample (N=2048 points). Host-side (inside kernel()):
weight preprocessing + batch sharding; device: kNN graph build (PE distance
matmul + int32-encoded single-pass top-k on DVE), edge-conv via the max/affine
decomposition
  max_k lrelu(bn(W @ [nbr - c; c])) == lrelu(bn(max_k(Wd @ x_nbr) + (Wc-Wd) @ x_c))
(exact: bn scale > 0 and lrelu are monotone), global max-pool, FC head.

Top-k scheme: psum pd is prescaled (S*v + 2^19 baked into the rhs), ACT
truncates psum -> int32 q, DVE computes enc = (q << 11) | col_iota, then a
single max8 pass per 256-col group + 3x(max8+match_replace) level-2 gives the
top-20 columns with indices in the low 11 bits (monotone int32-as-fp32
bitcast ordering).
"""

import numpy as np
import concourse.bass as bass
import concourse.bacc as bacc
import concourse.mybir as mybir
from concourse.tile import TileContext
from concourse import masks
from concourse.bass_utils import run_bass_kernel_spmd

dt = mybir.dt
AF = mybir.ActivationFunctionType
AX = mybir.AxisListType
ALU = mybir.AluOpType

EPS = 1e-5
SLOPE = 0.2
B, N, KNN = 8, 2048, 20
NT = N // 128  # 16 row tiles
WRAPB = 4      # tiles per wrap batch

ENC_B = 524288.0  # 2^19 bias on quantized distances
# per-layer quantization scales: S = (2^19 - 2^13) / R, R = safe |v| bound
SCALES = [(524288.0 - 8192.0) / 256.0,
          (524288.0 - 8192.0) / 1024.0,
          (524288.0 - 8192.0) / 1024.0]

_CACHE = {}


def _build(variant="full"):
    nq = 4 if variant == "q4big" else 1
    nc = bacc.Bacc("TRN2", target_bir_lowering=False, debug=False, num_devices=8,
                   num_swdge_queues=nq)

    # ---- I/O ----
    feat1_in = nc.dram_tensor("feat1", [4, N], dt.float32, kind="ExternalInput")
    rhspd1_in = nc.dram_tensor("rhspd1", [4, N], dt.float32, kind="ExternalInput")
    uvw_in = [
        nc.dram_tensor("uvw1", [3, 128], dt.float32, kind="ExternalInput"),
        nc.dram_tensor("uvw2", [64, 128], dt.float32, kind="ExternalInput"),
        nc.dram_tensor("uvw3", [64, 256], dt.float32, kind="ExternalInput"),
    ]
    sc_in = [
        nc.dram_tensor("sc1", [64, 1], dt.float32, kind="ExternalInput"),
        nc.dram_tensor("sc2", [64, 1], dt.float32, kind="ExternalInput"),
        nc.dram_tensor("sc3", [128, 1], dt.float32, kind="ExternalInput"),
    ]
    bi_in = [
        nc.dram_tensor("bi1", [64, 1], dt.float32, kind="ExternalInput"),
        nc.dram_tensor("bi2", [64, 1], dt.float32, kind="ExternalInput"),
        nc.dram_tensor("bi3", [128, 1], dt.float32, kind="ExternalInput"),
    ]
    w4t_in = nc.dram_tensor("w4t", [4, 64, 8, 128], dt.bfloat16, kind="ExternalInput")
    s4_in = nc.dram_tensor("s4", [128, 8], dt.float32, kind="ExternalInput")
    b4_in = nc.dram_tensor("b4", [128, 8], dt.float32, kind="ExternalInput")
    l1t_in = nc.dram_tensor("l1t", [8, 128, 4, 128], dt.float32, kind="ExternalInput")
    s5_in = nc.dram_tensor("s5", [128, 4], dt.float32, kind="ExternalInput")
    b5_in = nc.dram_tensor("b5", [128, 4], dt.float32, kind="ExternalInput")
    l2t_in = nc.dram_tensor("l2t", [128, 1024], dt.float32, kind="ExternalInput")
    s6_in = nc.dram_tensor("s6", [128, 2], dt.float32, kind="ExternalInput")
    b6_in = nc.dram_tensor("b6", [128, 2], dt.float32, kind="ExternalInput")
    l3t_in = nc.dram_tensor("l3t", [128, 4], dt.float32, kind="ExternalInput")
    l3b_in = nc.dram_tensor("l3b", [2, 1], dt.float32, kind="ExternalInput")

    out_t = nc.dram_tensor("out", [2, 1], dt.float32, kind="ExternalOutput")

    # ---- internal DRAM scratch ----
    idxflat_d = [
        nc.dram_tensor(f"idxflat{i}", [N * KNN], dt.int16, kind="Internal")
        for i in range(3)
    ]
    dram2_d = [
        nc.dram_tensor(f"idxrep{i}", [N * KNN // 16, 128], dt.int16, kind="Internal")
        for i in range(3)
    ]

    with TileContext(nc) as tc:
        with (
            tc.tile_pool(name="const", bufs=1) as cp,
            tc.tile_pool(name="feat", bufs=1) as fp,
            tc.tile_pool(name="work", bufs=2) as wp,
            tc.tile_pool(name="pd", bufs=3, space="PSUM") as psb,
            tc.tile_pool(name="pssm", bufs=2, space="PSUM") as pss,
        ):
            ident = cp.tile([128, 128], dt.float32)
            masks.make_identity(nc, ident[:])
            onesc = cp.tile([128, 1], dt.float32)
            nc.vector.memset(onesc[:, :], 1.0)
            iota = cp.tile([128, N], dt.int32)
            nc.gpsimd.iota(iota[:, :], pattern=[[1, N]], base=0,
                           channel_multiplier=0)
            encb_t = cp.tile([1, 1], dt.float32)
            nc.vector.memset(encb_t[:, :], ENC_B)
            sh11 = cp.tile([128, 1], dt.int32)
            nc.vector.memset(sh11[:, :], 11)
            mk2047 = cp.tile([128, 1], dt.int32)
            nc.vector.memset(mk2047[:, :], 2047)

            # load weights
            uvw_sb = [cp.tile(list(t.shape), dt.float32, name=f"uvw{i}", tag=f"uvw{i}") for i, t in enumerate(uvw_in)]
            sc_sb = [cp.tile(list(t.shape), dt.float32, name=f"sc{i}", tag=f"sc{i}") for i, t in enumerate(sc_in)]
            bi_sb = [cp.tile(list(t.shape), dt.float32, name=f"bi{i}", tag=f"bi{i}") for i, t in enumerate(bi_in)]
            for sb, inp in zip(uvw_sb + sc_sb + bi_sb, uvw_in + sc_in + bi_in):
                nc.sync.dma_start(sb[:, :], inp[:, :])
            s4_sb = cp.tile([128, 8], dt.float32)
            b4_sb = cp.tile([128, 8], dt.float32)
            s5_sb = cp.tile([128, 4], dt.float32)
            b5_sb = cp.tile([128, 4], dt.float32)
            l2t_sb = cp.tile([128, 1024], dt.float32)
            s6_sb = cp.tile([128, 2], dt.float32)
            b6_sb = cp.tile([128, 2], dt.float32)
            l3t_sb = cp.tile([128, 4], dt.float32)
            l3b_sb = cp.tile([2, 1], dt.float32)
            for sb, inp in [(s4_sb, s4_in), (b4_sb, b4_in), (s5_sb, s5_in),
                            (b5_sb, b5_in), (l2t_sb, l2t_in), (s6_sb, s6_in),
                            (b6_sb, b6_in), (l3t_sb, l3t_in), (l3b_sb, l3b_in)]:
                nc.sync.dma_start(sb[:, :], inp[:, :])

            # feature storage (rows 0..C-1 = features, row C = ones for pd matmul)
            feat1 = fp.tile([4, N], dt.float32)
            nc.sync.dma_start(feat1[:, :], feat1_in[:, :])
            feat2 = fp.tile([65, N], dt.float32)
            feat3 = fp.tile([65, N], dt.float32)
            x3a = fp.tile([64, N], dt.float32)
            x3b = fp.tile([64, N], dt.float32)
            rhs_pd = fp.tile([65, N], dt.float32)
            nc.sync.dma_start(rhs_pd[0:4, :], rhspd1_in[:, :])
            vfull = fp.tile([128, N], dt.float32)        # V^T (ch-major), per layer
            ut_sbuf = fp.tile([128, N], dt.bfloat16)     # U rows: point i at part i%128, stripe i//128
            hvec = fp.tile([128, 8], dt.float32)
            f1 = fp.tile([128, 4], dt.float32)
            f2 = fp.tile([128, 2], dt.float32)

            for li, feat in enumerate([feat1, feat2, feat3]):
                C = 3 if li == 0 else 64
                O = 128 if li == 2 else 64
                S = SCALES[li]
                lrhs = rhs_pd[0 : C + 1, :]

                if li > 0:
                    # ones row + rhs_pd build: [S*F ; -0.5*S*colsum(F^2) + ENC_B]
                    nc.gpsimd.memset(feat[C : C + 1, :], 1.0)
                    fsq = wp.tile([64, N], dt.float32, tag="fsq")
                    nc.scalar.square(fsq[0:C, :], feat[0:C, :])
                    nc.scalar.mul(rhs_pd[0:C, :], feat[0:C, :], S)
                    for j in range(4):
                        ps_sq = pss.tile([1, 512], dt.float32, tag="sm")
                        nc.tensor.matmul(ps_sq[:, :], onesc[0:C, 0:1],
                                         fsq[0:C, j * 512:(j + 1) * 512],
                                         start=True, stop=True)
                        nc.scalar.activation(rhs_pd[C : C + 1, j * 512:(j + 1) * 512],
                                             ps_sq[:, :], AF.Identity,
                                             bias=encb_t[:, 0:1], scale=-0.5 * S)

                # ---- stage B: U point-major (SBUF gather source) + V ch-major ----
                for t in range(NT):
                    ps_a = pss.tile([128, O], dt.float32, tag="sm")
                    nc.tensor.matmul(ps_a[:, :], feat[0:C, t * 128:(t + 1) * 128],
                                     uvw_sb[li][:, 0:O], start=True, stop=True)
                    nc.scalar.copy(ut_sbuf[:, t * 128:t * 128 + O], ps_a[:, :])
                    ps_b = pss.tile([O, 128], dt.float32, tag="sm")
                    nc.tensor.matmul(ps_b[:, :], uvw_sb[li][:, O:2 * O],
                                     feat[0:C, t * 128:(t + 1) * 128],
                                     start=True, stop=True)
                    nc.scalar.copy(vfull[0:O, t * 128:(t + 1) * 128], ps_b[:, :])

                # ---- stage C: pd + int-encoded topk, interleaved gathers ----
                for t in range(NT):
                    q = wp.tile([128, N], dt.int32, tag="q")
                    enc = wp.tile([128, N], dt.int32, tag="enc")
                    for h in range(2):
                        ps_pd = psb.tile([128, 1024], dt.float32, tag="big")
                        for j in range(2):
                            c0 = h * 1024 + j * 512
                            nc.tensor.matmul(ps_pd[:, j * 512:(j + 1) * 512],
                                             feat[0 : C + 1, t * 128:(t + 1) * 128],
                                             lrhs[:, c0:c0 + 512],
                                             start=True, stop=True)
                        # psum already holds S*v + ENC_B; trunc to int32
                        nc.scalar.copy(q[:, h * 1024:(h + 1) * 1024], ps_pd[:, :])
                        if variant != "notopk":
                            nc.vector.scalar_tensor_tensor(
                                out=enc[:, h * 1024:(h + 1) * 1024],
                                in0=q[:, h * 1024:(h + 1) * 1024], scalar=sh11[:, 0:1],
                                in1=iota[:, h * 1024:(h + 1) * 1024],
                                op0=ALU.arith_shift_left, op1=ALU.bitwise_or)

                    if variant != "notopk":
                        encf = enc[:, :].bitcast(dt.float32)
                        cand = wp.tile([128, 64], dt.float32, tag="cand")
                        for g in range(8):
                            nc.vector.max(out=cand[:, g * 8:(g + 1) * 8],
                                          in_=encf[:, g * 256:(g + 1) * 256])
                        m8 = wp.tile([128, 24], dt.float32, tag="m8")
                        c2 = wp.tile([128, 64], dt.float32, tag="csml")
                        c3 = wp.tile([128, 64], dt.float32, tag="csml")
                        nc.vector.max(out=m8[:, 0:8], in_=cand[:, :])
                        nc.vector.match_replace(out=c2[:, :], in_to_replace=m8[:, 0:8],
                                                in_values=cand[:, :], imm_value=0.0)
                        nc.vector.max(out=m8[:, 8:16], in_=c2[:, :])
                        nc.vector.match_replace(out=c3[:, :], in_to_replace=m8[:, 8:16],
                                                in_values=c2[:, :], imm_value=0.0)
                        nc.vector.max(out=m8[:, 16:24], in_=c3[:, :])
                    else:
                        m8 = wp.tile([128, 24], dt.float32, tag="m8")
                        nc.vector.tensor_copy(m8[:, :], q[:, 0:24].bitcast(dt.float32))

                    ps_it = pss.tile([24, 128], dt.float32, tag="sm")
                    nc.tensor.transpose(ps_it[:, :], m8[:, :], ident[:, :])
                    it32 = wp.tile([24, 128], dt.int32, tag="it32")
                    nc.vector.tensor_scalar(out=it32[:, :],
                                            in0=ps_it[:, :].bitcast(dt.int32),
                                            scalar1=mk2047[0:24, 0:1], scalar2=None,
                                            op0=ALU.bitwise_and)
                    it16 = wp.tile([24, 128], dt.int16, tag="it16")
                    nc.vector.tensor_copy(it16[:, :], it32[:, :])
                    nc.sync.dma_start(
                        idxflat_d[li].ap()[t * 2560:(t + 1) * 2560].rearrange("(k p) -> k p", p=128),
                        it16[0:KNN, :])

                    if t % WRAPB == WRAPB - 1:
                        hb = t // WRAPB
                        nrep = WRAPB * 2560 // 16  # 640 rows per batch
                        rsrc = idxflat_d[li].ap()[hb * WRAPB * 2560:(hb + 1) * WRAPB * 2560].rearrange(
                            "(j one pl) -> j one pl", pl=16, one=1).to_broadcast([nrep, 8, 16])
                        rdst = dram2_d[li].ap()[hb * nrep:(hb + 1) * nrep, :].rearrange(
                            "j (c pl) -> j c pl", pl=16)
                        nc.sync.dma_start(rdst, rsrc)
                        w = wp.tile([128, nrep], dt.int16, name="wrap", tag="wrap", bufs=2)
                        if variant == "notrdma":
                            nc.sync.dma_start(w[:, :], dram2_d[li].ap()[hb * nrep:(hb + 1) * nrep, :].rearrange("a (b c) -> (a b) c", b=128//16)[0:128, 0:nrep])
                        else:
                            nc.scalar.dma_start_transpose(w[:, :], dram2_d[li].ap()[hb * nrep:(hb + 1) * nrep, :])

                        # SBUF-source transpose gather + edge-feature finish
                        for tt in range(hb * WRAPB, (hb + 1) * WRAPB):
                            wrap = w[:, (tt % WRAPB) * 160:(tt % WRAPB + 1) * 160]
                            gout = wp.tile([128, KNN * 128], dt.bfloat16, tag="gout", bufs=3)
                            nc.gpsimd.dma_gather(
                                out_ap=gout[:].rearrange("c (j i) -> c j i", j=1),
                                in_ap=ut_sbuf[:, :],
                                idxs_ap=wrap,
                                num_idxs=KNN * 128,
                                num_idxs_reg=KNN * 128,
                                elem_size=128,
                                transpose=True,
                                single_packet=False,
                                sbuf_tokens_per_rank=128,
                                sbuf_free_dim_per_rank=256,
                            )
                            # tree-fold max over k=20 (column i = k*128 + pt)
                            fA = wp.tile([128, 1280], dt.bfloat16, tag="fA")
                            fB = wp.tile([128, 640], dt.bfloat16, tag="fB")
                            fC = wp.tile([128, 256], dt.bfloat16, tag="fC")
                            fD = wp.tile([128, 128], dt.bfloat16, tag="fD")
                            nc.vector.tensor_max(fA[0:O, :], gout[0:O, 0:1280], gout[0:O, 1280:2560])
                            nc.vector.tensor_max(fB[0:O, :], fA[0:O, 0:640], fA[0:O, 640:1280])
                            nc.vector.tensor_max(fC[0:O, :], fB[0:O, 0:256], fB[0:O, 256:512])
                            nc.vector.tensor_max(fD[0:O, :], fC[0:O, 0:128], fC[0:O, 128:256])
                            km = wp.tile([128, 128], dt.float32, tag="km")
                            nc.vector.tensor_max(km[0:O, :], fD[0:O, :], fB[0:O, 512:640])
                            hT = wp.tile([128, 128], dt.float32, tag="hT")
                            nc.gpsimd.tensor_add(hT[0:O, :], km[0:O, :],
                                                 vfull[0:O, tt * 128:(tt + 1) * 128])
                            cols = slice(tt * 128, (tt + 1) * 128)
                            if li == 0:
                                nc.scalar.activation(feat2[0:64, cols], hT[0:64, :],
                                                     AF.Prelu, bias=bi_sb[0][:, 0:1],
                                                     scale=sc_sb[0][:, 0:1], alpha=SLOPE)
                            elif li == 1:
                                nc.scalar.activation(feat3[0:64, cols], hT[0:64, :],
                                                     AF.Prelu, bias=bi_sb[1][:, 0:1],
                                                     scale=sc_sb[1][:, 0:1], alpha=SLOPE)
                            else:
                                zt = wp.tile([128, 128], dt.float32, tag="zt")
                                nc.scalar.activation(zt[:, :], hT[:, :],
                                                     AF.Prelu, bias=bi_sb[2][:, 0:1],
                                                     scale=sc_sb[2][:, 0:1], alpha=SLOPE)
                                nc.scalar.copy(x3a[:, cols], zt[0:64, :])
                                nc.vector.tensor_copy(x3b[:, cols], zt[64:128, :])

            # ---- stage D: W4 conv + bn4 + lrelu + global max (bf16) ----
            fb4 = []
            for src in [feat2, feat3, x3a, x3b]:
                fb = fp.tile([64, N], dt.bfloat16, name="fb4", tag="fb4", bufs=4)
                nc.scalar.copy(fb[:, :], src[0:64, :])
                fb4.append(fb)
            for t in range(8):
                wch = []
                for c4 in range(4):
                    w = wp.tile([64, 128], dt.bfloat16, name="wch4", tag="wch", bufs=8)
                    nc.sync.dma_start(w[:, :], w4t_in.ap()[c4, :, t, :])
                    wch.append(w)
                ymh = []
                for h in range(2):
                    ps_y = psb.tile([128, 1024], dt.float32, tag="big")
                    for j in range(2):
                        c0 = h * 1024 + j * 512
                        for c4 in range(4):
                            nc.tensor.matmul(ps_y[:, j * 512:(j + 1) * 512],
                                             wch[c4][:, :],
                                             fb4[c4][:, c0:c0 + 512],
                                             start=(c4 == 0), stop=(c4 == 3))
                    ym = wp.tile([128, 1], dt.float32, tag="ym", bufs=4)
                    nc.vector.reduce_max(ym[:, :], ps_y[:, :], axis=AX.X)
                    ymh.append(ym)
                ymc = wp.tile([128, 1], dt.float32, tag="ymc")
                nc.vector.tensor_max(ymc[:, :], ymh[0][:, :], ymh[1][:, :])
                nc.scalar.activation(hvec[:, t:t + 1], ymc[:, :], AF.Prelu,
                                     bias=b4_sb[:, t:t + 1], scale=s4_sb[:, t:t + 1],
                                     alpha=SLOPE)

            # ---- FC head ----
            for jt in range(4):
                ps_f = pss.tile([128, 1], dt.float32, tag="sm")
                lch = []
                for c in range(8):
                    w = wp.tile([128, 128], dt.float32, name="wchl", tag="wch", bufs=8)
                    nc.sync.dma_start(w[:, :], l1t_in.ap()[c, :, jt, :])
                    lch.append(w)
                for c in range(8):
                    nc.tensor.matmul(ps_f[:, :], lch[c][:, :], hvec[:, c:c + 1],
                                     start=(c == 0), stop=(c == 7))
                nc.scalar.activation(f1[:, jt:jt + 1], ps_f[:, :], AF.Prelu,
                                     bias=b5_sb[:, jt:jt + 1], scale=s5_sb[:, jt:jt + 1],
                                     alpha=SLOPE)

            for jt in range(2):
                ps_f = pss.tile([128, 1], dt.float32, tag="sm")
                for c in range(4):
                    nc.tensor.matmul(ps_f[:, :],
                                     l2t_sb[:, (c * 2 + jt) * 128:(c * 2 + jt + 1) * 128],
                                     f1[:, c:c + 1], start=(c == 0), stop=(c == 3))
                nc.scalar.activation(f2[:, jt:jt + 1], ps_f[:, :], AF.Prelu,
                                     bias=b6_sb[:, jt:jt + 1], scale=s6_sb[:, jt:jt + 1],
                                     alpha=SLOPE)

            ps_o = pss.tile([2, 1], dt.float32, tag="sm")
            for c in range(2):
                nc.tensor.matmul(ps_o[:, :], l3t_sb[:, c * 2:(c + 1) * 2],
                                 f2[:, c:c + 1], start=(c == 0), stop=(c == 1))
            osb = wp.tile([2, 1], dt.float32, tag="osb")
            nc.scalar.activation(osb[:, :], ps_o[:, :], AF.Identity,
                                 bias=l3b_sb[:, 0:1], scale=1.0)
            nc.sync.dma_start(out_t.ap()[:, :], osb[:, :])

    nc.compile()
    return nc


def _prep_weights(w):
    f32 = np.float32

    def bn_fold(g, b, m, v):
        s = (g / np.sqrt(v + EPS)).astype(f32)
        return s, (b - m * s).astype(f32)

    out = {}
    W1, W2, W3 = w["W1"], w["W2"], w["W3"]
    for i, (W, C) in enumerate([(W1, 3), (W2, 64), (W3, 64)]):
        wd, wc = W[:, :C], W[:, C:]
        out[f"uvw{i+1}"] = np.concatenate([wd.T, (wc - wd).T], axis=1).astype(f32).copy()
        s, b = bn_fold(w[f"bn{i+1}_g"], w[f"bn{i+1}_b"], w[f"bn{i+1}_m"], w[f"bn{i+1}_v"])
        out[f"sc{i+1}"] = s.reshape(-1, 1).copy()
        out[f"bi{i+1}"] = b.reshape(-1, 1).copy()

    import ml_dtypes
    out["w4t"] = w["W4"].T.reshape(4, 64, 8, 128).astype(ml_dtypes.bfloat16).copy()  # (256,1024)->(c4,64,t,128)
    s4, b4 = bn_fold(w["bn4_g"], w["bn4_b"], w["bn4_m"], w["bn4_v"])
    out["s4"] = s4.reshape(8, 128).T.copy()
    out["b4"] = b4.reshape(8, 128).T.copy()

    l1t = w["l1_w"].T  # (1024, 512)
    out["l1t"] = l1t.reshape(8, 128, 4, 128).copy()
    s5, b5 = bn_fold(w["bn5_g"], w["bn5_b"], w["bn5_m"], w["bn5_v"])
    b5e = (s5 * w["l1_b"] + b5).astype(f32)
    out["s5"] = s5.reshape(4, 128).T.copy()
    out["b5"] = b5e.reshape(4, 128).T.copy()

    l2t = w["l2_w"].T  # (512, 256)
    out["l2t"] = l2t.reshape(4, 128, 2, 128).transpose(1, 0, 2, 3).reshape(128, 1024).copy()
    s6, b6 = bn_fold(w["bn6_g"], w["bn6_b"], w["bn6_m"], w["bn6_v"])
    b6e = (s6 * w["l2_b"] + b6).astype(f32)
    out["s6"] = s6.reshape(2, 128).T.copy()
    out["b6"] = b6e.reshape(2, 128).T.copy()

    out["l3t"] = w["l3_w"].T.reshape(2, 128, 2).transpose(1, 0, 2).reshape(128, 4).copy()
    out["l3b"] = w["l3_b"].reshape(2, 1).copy()
    return out


def make_in_maps(inputs):
    x = np.asarray(inputs["x"], dtype=np.float32)
    wmaps = _prep_weights({k: np.asarray(v, dtype=np.float32) for k, v in inputs.items() if k != "x"})
    S0 = SCALES[0]
    in_maps = []
    for i in range(B):
        xt = x[i].T.copy()  # (3, 2048)
        sumsq = (xt * xt).sum(axis=0, keepdims=True)
        m = dict(wmaps)
        m["feat1"] = np.concatenate([xt, np.ones((1, N), np.float32)], axis=0).copy()
        m["rhspd1"] = np.concatenate(
            [S0 * xt, -0.5 * S0 * sumsq + ENC_B], axis=0).astype(np.float32).copy()
        in_maps.append(m)
    return in_maps


def kernel(**inputs):
    if "nc" not in _CACHE:
        _CACHE["nc"] = _build()
    nc = _CACHE["nc"]

    in_maps = make_in_maps(inputs)
    res = run_bass_kernel_spmd(nc, in_maps, core_ids=list(range(B)))
    out = np.stack([res.results[i]["out"].reshape(2) for i in range(B)], axis=0)
    return out.astype(np.float32)


def make_timed_runner(nc, in_maps, reps=1):
    """Build a sharded jit callable that executes the NEFF once; run()
    dispatches it `reps` times back-to-back (async, queued in submission
    order on the device) and blocks at the end. Returns run() -> wall s."""
    import jax
    import jax.numpy as jnp
    import time
    from jax.sharding import Mesh, PartitionSpec, NamedSharding
    from jax.experimental.shard_map import shard_map
    import concourse.mybir as mb
    from concourse import bass2jax
    from concourse.bass2jax import _bass_exec_p, partition_id_tensor

    bass2jax.install_neuronx_cc_hook()
    n_cores = len(in_maps)
    partition_name = nc.partition_id_tensor.name if nc.partition_id_tensor else None
    in_names, out_names, out_avals, zero_outs = [], [], [], []
    for alloc in nc.m.functions[0].allocations:
        if not isinstance(alloc, mb.MemoryLocationSet):
            continue
        name = alloc.memorylocations[0].name
        if alloc.kind == "ExternalInput":
            if name != partition_name:
                in_names.append(name)
        elif alloc.kind == "ExternalOutput":
            shape = tuple(alloc.tensor_shape)
            dtype = mb.dt.np(alloc.dtype)
            out_names.append(name)
            out_avals.append(jax.core.ShapedArray(shape, dtype))
            zero_outs.append(np.zeros(shape, dtype))
    n_params = len(in_names)
    n_outs = len(out_avals)
    names_all = tuple(in_names + out_names + ([partition_name] if partition_name else []))

    def _body(*args):
        operands = list(args)
        if partition_name is not None:
            operands.append(partition_id_tensor())
        outs = _bass_exec_p.bind(
            *operands,
            out_avals=tuple(out_avals),
            in_names=names_all,
            out_names=tuple(out_names),
            lowering_input_output_aliases=(),
            sim_require_finite=True,
            sim_require_nnan=True,
            nc=nc,
        )
        return tuple(outs)

    devices = jax.devices()[:n_cores]
    mesh = Mesh(np.asarray(devices), ("core",))
    in_specs = (PartitionSpec("core"),) * (n_params + n_outs)
    out_specs = (PartitionSpec("core"),) * n_outs
    sharded = jax.jit(
        shard_map(_body, mesh=mesh, in_specs=in_specs, out_specs=out_specs, check_rep=False),
        keep_unused=True,
    )
    sh = NamedSharding(mesh, PartitionSpec("core"))
    concat_np = {
        k: np.concatenate([np.asarray(in_maps[c][k]) for c in range(n_cores)], axis=0)
        for k in in_names
    }
    concat_in = [jax.device_put(concat_np[k], sh) for k in in_names]
    czeros = [
        jax.device_put(np.zeros((n_cores * z.shape[0], *z.shape[1:]), z.dtype), sh)
        for z in zero_outs
    ]

    def run():
        t0 = time.perf_counter()
        out = None
        for _ in range(reps):
            out = sharded(*concat_in, *czeros)
        jax.block_until_ready(out)
        return time.perf_counter() - t0

    return run


if __name__ == "__main__":
    import reference

    inputs = {k: np.asarray(v) for k, v in reference.setup_inputs().items()}
    got = kernel(**inputs)
    exp = np.asarray(reference.reference(**inputs))
    err = np.abs(got - exp).max() / max(np.abs(exp).max(), 1e-9)
    print("got:\n", got)
    print("exp:\n", exp)
    print("rel err:", err)
